# revision 2
# baseline (speedup 1.0000x reference)
"""Butterfly network forward pass on 8 Trainium2 NeuronCores.

Strategy (v2): split the 10 butterfly stages at the 256-feature boundary.
Stages 0-7 (strides 1..128) only mix features within aligned 256-blocks,
so they compose into a block-diagonal matrix with four 256x256 blocks —
a GEMM with contraction 256 instead of the dense 1024 (4x fewer PE MACs
than the v1 dense fold).  Stages 8-9 (strides 256/512) mix feature
chunks c and c+2 / c+4 at a fixed within-chunk position p: with features
laid out as 8 chunks of 128 on the SBUF partition axis, their 2x2
twiddles become per-partition scalars, so each stage is one fused
scalar_tensor_tensor per output chunk on the Vector engine.

Per core (batch shard 2048 rows, everything fp16 on the wire):
  PE:  4 batch-chunks x 8 out-chunks x 2 accumulated [128x128]x[128x512]
       matmuls (fp16, fp32 PSUM)               ~14 us
  ACT: PSUM->SBUF evacuation, fp16 out, with the stage-8 "own"
       coefficient folded into the per-partition activation scale
  DVE: stage 8 as z_a = e_a + (t01/t11) (.) e_b   (one STT per chunk)
       stage 9 as u = (z (.) t9a) + bias; out = (z' (.) t9b) + u
  DMA: in 4 MB x + 0.5 MB weights, out 4 MB      ~24 us  <- roofline

The host folds stages 0-7 into the block matrices, precomputes the
stage-8/9 per-partition coefficient vectors (ratios guarded against
tiny denominators), and converts x to fp16; the device returns out^T
in fp16 which the host transposes/upcasts while gathering.
"""

import numpy as np

import concourse.bacc as bacc
import concourse.mybir as mybir
import concourse.tile as tile
from concourse.bass_utils import run_bass_kernel_spmd

N_CORES = 8
BATCH = 16384
N = 1024
M_STAGES = 10
SHARD = BATCH // N_CORES   # 2048 rows per core
P = 128                    # SBUF partitions
NB = 512                   # moving-dim (batch) chunk per matmul
NBC = SHARD // NB          # 4 batch chunks per core
NCH = N // P               # 8 feature chunks of 128

F32 = mybir.dt.float32
F16 = mybir.dt.float16
IDENT = mybir.ActivationFunctionType.Identity
MULT = mybir.AluOpType.mult
ADD = mybir.AluOpType.add

S8_PAIRS = [(0, 2), (1, 3), (4, 6), (5, 7)]

_NC_CACHE = None


def build_nc(reps: int = 1):
    """Build the per-core kernel; reps>1 wraps the FULL body (including
    all input DMA) in a hardware loop for A/B timing by subtraction."""
    nc = bacc.Bacc("TRN2", target_bir_lowering=False, debug=False,
                   num_devices=N_CORES)
    xT = nc.declare_dram_parameter("xT", [N, SHARD], F16, isOutput=False)
    # lhsT blocks: column block (m*2+k) holds A^T rows for out-chunk m,
    # contraction half k: wB[p, (m*2+k)*P + q] = A[m*128+q, (m//2)*256+k*128+p]
    wB = nc.declare_dram_parameter("wB", [P, 2 * NCH * P], F16, isOutput=False)
    # per-partition coefficient columns: [alpha | r8 | t9a | t9b | bias]
    coef = nc.declare_dram_parameter("coef", [P, 5 * NCH], F32, isOutput=False)
    outT = nc.declare_dram_parameter("outT", [N, SHARD], F16, isOutput=True)

    with tile.TileContext(nc) as tc:
        with (
            tc.tile_pool(name="wp", bufs=1) as wp,
            tc.tile_pool(name="xp", bufs=1) as xp,
            tc.tile_pool(name="cp", bufs=1) as cp,
            tc.tile_pool(name="pp", bufs=7, space="PSUM") as pp,
            tc.tile_pool(name="ppw", bufs=1, space="PSUM") as ppw,
            tc.tile_pool(name="ep", bufs=2) as ep,
            tc.tile_pool(name="zp", bufs=2) as zp,
            tc.tile_pool(name="up", bufs=4) as up,
            tc.tile_pool(name="op", bufs=8) as op,
        ):
            def body():
                ct = cp.tile([P, 5 * NCH], F32, tag="coef")
                nc.sync.dma_start(out=ct[:], in_=coef[:])
                alpha = ct[:, 0 * NCH:1 * NCH]
                r8 = ct[:, 1 * NCH:2 * NCH]
                t9a = ct[:, 2 * NCH:3 * NCH]
                t9b = ct[:, 3 * NCH:4 * NCH]
                bias8 = ct[:, 4 * NCH:5 * NCH]

                wt = wp.tile([P, 2 * NCH * P], F16, tag="w")
                nc.sync.dma_start(out=wt[:], in_=wB[:])

                # x chunk tiles: column block k holds xT[k*P:(k+1)*P,
                # n*NB:(n+1)*NB]
                xsrc = xT.rearrange("(k p) (nb b) -> nb p k b", p=P, b=NB)
                xtiles = [xp.tile([P, NCH * NB], F16, tag=f"xc{n}",
                                  name=f"xc{n}") for n in range(NBC)]
                x0 = xtiles[0].rearrange("p (k b) -> p k b", b=NB)
                for k in range(NCH):
                    nc.sync.dma_start(out=x0[:, k], in_=xsrc[0, :, k])
                for n in range(1, NBC):
                    dst = xtiles[n][:].rearrange("p (k b) -> p k b", b=NB)
                    h = NCH // 2
                    nc.sync.dma_start(out=dst[:, 0:h], in_=xsrc[n, :, 0:h])
                    nc.sync.dma_start(out=dst[:, h:NCH], in_=xsrc[n, :, h:NCH])

                # Warm the PE (HAM clock gate) while the prologue streams.
                wps = ppw.tile([NCH, NCH], F32, tag="warm")
                for _ in range(16):
                    nc.tensor.matmul(wps[:], lhsT=ct[:, 0:NCH],
                                     rhs=ct[:, 0:NCH], start=True, stop=True)

                for n in range(NBC):
                    xt = xtiles[n]
                    etiles = []
                    for m in range(NCH):
                        c = m // 2
                        ps = pp.tile([P, NB], F32, tag="ps")
                        for kk in range(2):
                            nc.tensor.matmul(
                                ps[:],
                                lhsT=wt[:, (m * 2 + kk) * P:(m * 2 + kk + 1) * P],
                                rhs=xt[:, (2 * c + kk) * NB:(2 * c + kk + 1) * NB],
                                start=(kk == 0),
                                stop=(kk == 1),
                            )
                        # evacuate with the stage-8 diagonal coefficient
                        # folded into the ACT per-partition scale
                        et = ep.tile([P, NB], F16, tag=f"e{m}", name=f"e{m}")
                        nc.scalar.activation(et[:], ps[:], IDENT,
                                             scale=alpha[:, m:m + 1])
                        etiles.append(et)

                    # stage 8: z_a = e_a + r8_a (.) e_b ; z_b = e_b + r8_b (.) e_a
                    ztiles = [None] * NCH
                    for (a, b) in S8_PAIRS:
                        za = zp.tile([P, NB], F16, tag=f"z{a}", name=f"z{a}")
                        nc.vector.scalar_tensor_tensor(
                            za[:], etiles[b][:], r8[:, a:a + 1], etiles[a][:],
                            op0=MULT, op1=ADD)
                        zb = zp.tile([P, NB], F16, tag=f"z{b}", name=f"z{b}")
                        nc.vector.scalar_tensor_tensor(
                            zb[:], etiles[a][:], r8[:, b:b + 1], etiles[b][:],
                            op0=MULT, op1=ADD)
                        ztiles[a], ztiles[b] = za, zb

                    # stage 9 + bias: out_m = t9a_m (.) z_{m%4} +
                    #                          t9b_m (.) z_{m%4+4} + bias_m
                    for m in range(NCH):
                        zlo, zhi = ztiles[m % 4], ztiles[m % 4 + 4]
                        ut = up.tile([P, NB], F16, tag="u")
                        nc.vector.tensor_scalar(
                            ut[:], zlo[:], t9a[:, m:m + 1], bias8[:, m:m + 1],
                            op0=MULT, op1=ADD)
                        ot = op.tile([P, NB], F16, tag="ot")
                        nc.vector.scalar_tensor_tensor(
                            ot[:], zhi[:], t9b[:, m:m + 1], ut[:],
                            op0=MULT, op1=ADD)
                        nc.sync.dma_start(
                            out=outT[m * P:(m + 1) * P, n * NB:(n + 1) * NB],
                            in_=ot[:])

            if reps == 1:
                body()
            else:
                with tc.For_i(0, reps, 1):
                    body()
    nc.compile()
    return nc


def _clamp_den(v, eps=1e-20):
    s = np.where(v >= 0, 1.0, -1.0).astype(np.float32)
    return np.where(np.abs(v) < eps, s * eps, v).astype(np.float32)


def compose_low_stages(twiddle) -> np.ndarray:
    """Fold stages 0..7 into A^T = lowstages(I_N): [feat_in, feat_out],
    block-diagonal with four 256x256 blocks."""
    out = np.eye(N, dtype=np.float32)
    tw = np.asarray(twiddle, dtype=np.float32)
    for s in range(8):
        stride = 1 << s
        nblk = N // (2 * stride)
        t = tw[0, s].reshape(nblk, stride, 2, 2)
        xr = out.reshape(N, nblk, 2, stride)
        out = np.einsum("krij,bkjr->bkir", t, xr,
                        dtype=np.float32).reshape(N, N)
    return out


def make_coeffs(twiddle, bias):
    """Per-partition coefficient table [P, 5*NCH] fp32:
    columns [alpha | r8 | t9a | t9b | bias]."""
    tw = np.asarray(twiddle, dtype=np.float32)
    t8 = tw[0, 8].reshape(2, 256, 2, 2)   # [k8, r, i, j]
    t9 = tw[0, 9].reshape(512, 2, 2)      # [r, i, j]
    p = np.arange(P)
    alpha = np.zeros((P, NCH), np.float32)
    r8 = np.zeros((P, NCH), np.float32)
    t9a = np.zeros((P, NCH), np.float32)
    t9b = np.zeros((P, NCH), np.float32)
    for (a, b) in S8_PAIRS:
        k8 = a // 4
        r = (a % 2) * 128 + p
        t00 = _clamp_den(t8[k8, r, 0, 0])
        t01 = t8[k8, r, 0, 1]
        t10 = t8[k8, r, 1, 0]
        t11 = _clamp_den(t8[k8, r, 1, 1])
        alpha[:, a] = t00
        alpha[:, b] = t11
        r8[:, a] = t01 / t11      # z_a = e_a + r8_a * e_b
        r8[:, b] = t10 / t00      # z_b = e_b + r8_b * e_a
    for m in range(NCH):
        i = m // 4
        r = (m % 4) * 128 + p
        t9a[:, m] = t9[r, i, 0]   # coefficient of z_{m%4}
        t9b[:, m] = t9[r, i, 1]   # coefficient of z_{m%4+4}
    bias8 = np.asarray(bias, np.float32).reshape(NCH, P).T
    return np.ascontiguousarray(
        np.concatenate([alpha, r8, t9a, t9b, bias8], axis=1))


def make_weights(twiddle) -> np.ndarray:
    """lhsT blocks packed as [P, 2*NCH*P] fp16."""
    aT = compose_low_stages(twiddle)    # [in, out]
    w = np.empty((P, 2 * NCH * P), np.float16)
    for m in range(NCH):
        c = m // 2
        for kk in range(2):
            blk = aT[c * 256 + kk * 128: c * 256 + kk * 128 + 128,
                     m * 128:(m + 1) * 128]
            w[:, (m * 2 + kk) * P:(m * 2 + kk + 1) * P] = blk.astype(np.float16)
    return np.ascontiguousarray(w)


def make_inputs(x, twiddle, bias):
    wB = make_weights(twiddle)
    coef = make_coeffs(twiddle, bias)
    x = np.asarray(x)
    in_maps = []
    for c in range(N_CORES):
        shard = x[c * SHARD:(c + 1) * SHARD]
        in_maps.append({
            "xT": np.ascontiguousarray(shard.T.astype(np.float16)),
            "wB": wB,
            "coef": coef,
        })
    return in_maps


def kernel(x: np.ndarray, twiddle: np.ndarray, bias: np.ndarray) -> np.ndarray:
    global _NC_CACHE
    if _NC_CACHE is None:
        _NC_CACHE = build_nc()
    nc = _NC_CACHE

    in_maps = make_inputs(x, twiddle, bias)
    res = run_bass_kernel_spmd(nc, in_maps, list(range(N_CORES)))
    out = np.empty((BATCH, N), dtype=np.float32)
    for c in range(N_CORES):
        out[c * SHARD:(c + 1) * SHARD] = res.results[c]["outT"].T.astype(np.float32)
    return out


# revision 4
# speedup vs baseline: 1.3172x; 1.3172x over previous
"""Butterfly network forward pass on 8 Trainium2 NeuronCores.

Strategy (v3, mixed-radix split): stages 0-7 (strides 1..128) mix only
within aligned 256-blocks; stage 8 (stride 256) within 512-blocks;
stage 9 (stride 512) across them.  The kernel folds a DIFFERENT number
of stages into the GEMM per feature half to balance the engines:

  LO half (features 0-511):  stages 0-8 fold into one 512x512 matrix ->
      GEMM contraction 512 (4 accumulated matmuls), PSUM holds the
      stage-8 output z_0..z_3 directly.
  HI half (features 512-1023): stages 0-7 fold into two 256x256 blocks ->
      GEMM contraction 256 (2 matmuls) -> y_4..y_7; stage 8 runs on the
      Vector engine with per-partition twiddle scalars (the "own"
      coefficient is folded into the PSUM-evacuation scale on the Scalar
      engine, the cross term is a tensor_scalar + tensor_tensor pair).

  Stage 9 + bias (pairs z_m, z_{m+4} at fixed partition): per output
  chunk u = (z_lo * t9a + bias) on GpSimd, v = (z_hi * t9b) on DVE,
  out = u + v on DVE.  Stage ops run at free-dim 1024 (two batch chunks)
  to amortize the DVE errata bubble.

Per-core modeled engine busy (cost model == HW within 3%):
  PE 96 matmuls ~20.7us | ACT 32 evacs ~19.6us | DVE ~22.1us |
  GpSimd ~14us | DMA wire 8.6 MB ~23.8us  <- everything at the HBM floor

Everything crosses the wire in fp16 (x pre-cast on host, out upcast on
host); fp32 PSUM accumulation, fp32 per-partition coefficient tables.
"""

import numpy as np

import concourse.bacc as bacc
import concourse.mybir as mybir
import concourse.tile as tile
from concourse.bass_utils import run_bass_kernel_spmd

N_CORES = 8
BATCH = 16384
N = 1024
M_STAGES = 10
SHARD = BATCH // N_CORES   # 2048 rows per core
P = 128                    # SBUF partitions
NB = 512                   # matmul moving-dim chunk (one PSUM bank fp32)
NBC = SHARD // NB          # 4 batch chunks per core
NCH = N // P               # 8 feature chunks of 128
FDS = 1024                 # free dim for the SBUF stage ops
NGRP = SHARD // FDS        # 2 stage-op column groups

F32 = mybir.dt.float32
F16 = mybir.dt.float16
IDENT = mybir.ActivationFunctionType.Identity
MULT = mybir.AluOpType.mult
ADD = mybir.AluOpType.add

S8_PAIRS_HI = [(4, 6), (5, 7)]
# weight column-block index: LO (m,k) then HI (m,kk)
NWBLK = 4 * 4 + 4 * 2

_NC_CACHE = None


def build_nc(reps: int = 1):
    """Build the per-core kernel; reps>1 wraps the FULL body (including
    all input DMA) in a hardware loop for A/B timing by subtraction."""
    nc = bacc.Bacc("TRN2", target_bir_lowering=False, debug=False,
                   num_devices=N_CORES)
    xT = nc.declare_dram_parameter("xT", [N, SHARD], F16, isOutput=False)
    wB = nc.declare_dram_parameter("wB", [P, NWBLK * P], F16, isOutput=False)
    # per-partition coefficient columns: [alpha | r8 | t9a | t9b | bias]
    coef = nc.declare_dram_parameter("coef", [P, 5 * NCH], F32, isOutput=False)
    outT = nc.declare_dram_parameter("outT", [N, SHARD], F16, isOutput=True)

    with tile.TileContext(nc) as tc:
        with (
            tc.tile_pool(name="wp", bufs=2) as wp,
            tc.tile_pool(name="xp", bufs=2) as xp,
            tc.tile_pool(name="cp", bufs=2) as cp,
            tc.tile_pool(name="pp", bufs=7, space="PSUM") as pp,
            tc.tile_pool(name="ppw", bufs=1, space="PSUM") as ppw,
            tc.tile_pool(name="ep", bufs=2) as ep,
            tc.tile_pool(name="zp", bufs=2) as zp,
            tc.tile_pool(name="up", bufs=4) as up,
            tc.tile_pool(name="op", bufs=8) as op,
        ):
            def body():
                ct = cp.tile([P, 5 * NCH], F32, tag="coef")
                nc.sync.dma_start(out=ct[:], in_=coef[:])
                alpha = ct[:, 0 * NCH:1 * NCH]
                r8 = ct[:, 1 * NCH:2 * NCH]
                t9a = ct[:, 2 * NCH:3 * NCH]
                t9b = ct[:, 3 * NCH:4 * NCH]
                bias8 = ct[:, 4 * NCH:5 * NCH]

                wt = wp.tile([P, NWBLK * P], F16, tag="w")
                nc.sync.dma_start(out=wt[:], in_=wB[:])

                def wblk(i):
                    return wt[:, i * P:(i + 1) * P]

                # x chunk tiles: column block k holds xT[k*P:(k+1)*P,
                # n*NB:(n+1)*NB]
                xsrc = xT.rearrange("(k p) (nb b) -> nb p k b", p=P, b=NB)
                xtiles = [xp.tile([P, NCH * NB], F16, tag=f"xc{n}",
                                  name=f"xc{n}") for n in range(NBC)]
                x0 = xtiles[0].rearrange("p (k b) -> p k b", b=NB)
                for k in range(0, NCH, 2):
                    nc.sync.dma_start(out=x0[:, k:k + 2], in_=xsrc[0, :, k:k + 2])
                for n in range(1, NBC):
                    dst = xtiles[n][:].rearrange("p (k b) -> p k b", b=NB)
                    h = NCH // 2
                    nc.sync.dma_start(out=dst[:, 0:h], in_=xsrc[n, :, 0:h])
                    nc.sync.dma_start(out=dst[:, h:NCH], in_=xsrc[n, :, h:NCH])

                # Warm the PE (HAM clock gate) while the prologue streams.
                wps = ppw.tile([NCH, NCH], F32, tag="warm")
                for _ in range(16):
                    nc.tensor.matmul(wps[:], lhsT=ct[:, 0:NCH],
                                     rhs=ct[:, 0:NCH], start=True, stop=True)

                # e tiles: full chunk rows at stage-op width (two halves
                # written by two evacs each).  LO e == z directly.
                etiles = {}

                for g in range(NGRP):
                    for m in range(NCH):
                        etiles[(g, m)] = ep.tile([P, FDS], F16,
                                                 tag=f"e{m}", name=f"e{m}")
                    for n in (2 * g, 2 * g + 1):
                        xt = xtiles[n]
                        half = (n % 2) * NB
                        for m in range(NCH):
                            ps = pp.tile([P, NB], F32, tag="ps")
                            if m < 4:       # LO: contraction 512, stages 0-8
                                for k in range(4):
                                    nc.tensor.matmul(
                                        ps[:], lhsT=wblk(m * 4 + k),
                                        rhs=xt[:, k * NB:(k + 1) * NB],
                                        start=(k == 0), stop=(k == 3))
                            else:           # HI: contraction 256, stages 0-7
                                c = m // 2
                                for kk in range(2):
                                    nc.tensor.matmul(
                                        ps[:], lhsT=wblk(16 + (m - 4) * 2 + kk),
                                        rhs=xt[:, (2 * c + kk) * NB:
                                               (2 * c + kk + 1) * NB],
                                        start=(kk == 0), stop=(kk == 1))
                            et = etiles[(g, m)]
                            nc.scalar.activation(
                                et[:, half:half + NB], ps[:], IDENT,
                                scale=alpha[:, m:m + 1])

                    # stage 8 on the HI half
                    ztiles = {}
                    for (a, b) in S8_PAIRS_HI:
                        ea, eb = etiles[(g, a)], etiles[(g, b)]
                        ta = up.tile([P, FDS], F16, tag="t8")
                        nc.vector.tensor_scalar(ta[:], eb[:], r8[:, a:a + 1],
                                                None, op0=MULT)
                        za = zp.tile([P, FDS], F16, tag=f"z{a}", name=f"z{a}")
                        nc.vector.tensor_tensor(za[:], ta[:], ea[:], op=ADD)
                        tb = up.tile([P, FDS], F16, tag="t8")
                        nc.vector.tensor_scalar(tb[:], ea[:], r8[:, b:b + 1],
                                                None, op0=MULT)
                        zb = zp.tile([P, FDS], F16, tag=f"z{b}", name=f"z{b}")
                        nc.vector.tensor_tensor(zb[:], tb[:], eb[:], op=ADD)
                        ztiles[a], ztiles[b] = za, zb

                    # stage 9 + bias: out_m = t9a_m (.) z_lo + t9b_m (.) z_hi
                    # + bias_m with z_lo = e_{m%4} (LO fold), z_hi = z_{m%4+4}
                    for m in range(NCH):
                        zlo = etiles[(g, m % 4)]
                        zhi = ztiles[m % 4 + 4]
                        ut = up.tile([P, FDS], F16, tag="u")
                        nc.gpsimd.tensor_scalar(
                            ut[:], zlo[:], t9a[:, m:m + 1], bias8[:, m:m + 1],
                            op0=MULT, op1=ADD)
                        vt = up.tile([P, FDS], F16, tag="v")
                        nc.vector.tensor_scalar(vt[:], zhi[:], t9b[:, m:m + 1],
                                                None, op0=MULT)
                        ot = op.tile([P, FDS], F16, tag="ot")
                        nc.vector.tensor_tensor(ot[:], ut[:], vt[:], op=ADD)
                        nc.sync.dma_start(
                            out=outT[m * P:(m + 1) * P,
                                     g * FDS:(g + 1) * FDS],
                            in_=ot[:])

            if reps == 1:
                body()
            else:
                with tc.For_i(0, reps, 1):
                    body()
    nc.compile()
    return nc


def _clamp_den(v, eps=1e-20):
    s = np.where(v >= 0, 1.0, -1.0).astype(np.float32)
    return np.where(np.abs(v) < eps, s * eps, v).astype(np.float32)


def compose_stages(twiddle, hi) -> np.ndarray:
    """Fold stages 0..hi-1 into A^T = stages(I_N): [feat_in, feat_out]."""
    out = np.eye(N, dtype=np.float32)
    tw = np.asarray(twiddle, dtype=np.float32)
    for s in range(hi):
        stride = 1 << s
        nblk = N // (2 * stride)
        t = tw[0, s].reshape(nblk, stride, 2, 2)
        xr = out.reshape(N, nblk, 2, stride)
        out = np.einsum("krij,bkjr->bkir", t, xr,
                        dtype=np.float32).reshape(N, N)
    return out


def make_coeffs(twiddle, bias):
    """Per-partition coefficient table [P, 5*NCH] fp32:
    columns [alpha | r8 | t9a | t9b | bias]."""
    tw = np.asarray(twiddle, dtype=np.float32)
    t8 = tw[0, 8].reshape(2, 256, 2, 2)   # [k8, r, i, j]
    t9 = tw[0, 9].reshape(512, 2, 2)      # [r, i, j]
    p = np.arange(P)
    alpha = np.ones((P, NCH), np.float32)   # LO chunks evac unscaled
    r8 = np.zeros((P, NCH), np.float32)
    t9a = np.zeros((P, NCH), np.float32)
    t9b = np.zeros((P, NCH), np.float32)
    for (a, b) in S8_PAIRS_HI:
        k8 = a // 4                       # == 1
        r = (a % 2) * 128 + p
        t00 = _clamp_den(t8[k8, r, 0, 0])
        t01 = t8[k8, r, 0, 1]
        t10 = t8[k8, r, 1, 0]
        t11 = _clamp_den(t8[k8, r, 1, 1])
        alpha[:, a] = t00
        alpha[:, b] = t11
        r8[:, a] = t01 / t11              # z_a = e_a + r8_a * e_b
        r8[:, b] = t10 / t00              # z_b = e_b + r8_b * e_a
    for m in range(NCH):
        i = m // 4
        r = (m % 4) * 128 + p
        t9a[:, m] = t9[r, i, 0]           # coefficient of z_{m%4}
        t9b[:, m] = t9[r, i, 1]           # coefficient of z_{m%4+4}
    bias8 = np.asarray(bias, np.float32).reshape(NCH, P).T
    return np.ascontiguousarray(
        np.concatenate([alpha, r8, t9a, t9b, bias8], axis=1))


def make_weights(twiddle) -> np.ndarray:
    """lhsT blocks packed as [P, NWBLK*P] fp16: 16 LO blocks (contraction
    512, stages 0-8) then 8 HI blocks (contraction 256, stages 0-7)."""
    aT8 = compose_stages(twiddle, 8)      # [in, out], 256-block diagonal
    aT9 = compose_stages(twiddle, 9)      # [in, out], 512-block diagonal
    w = np.empty((P, NWBLK * P), np.float16)
    for m in range(4):                    # LO out-chunks
        for k in range(4):
            blk = aT9[k * P:(k + 1) * P, m * P:(m + 1) * P]
            w[:, (m * 4 + k) * P:(m * 4 + k + 1) * P] = blk.astype(np.float16)
    for m in range(4, NCH):               # HI out-chunks
        c = m // 2
        for kk in range(2):
            fin = c * 256 + kk * P
            blk = aT8[fin:fin + P, m * P:(m + 1) * P]
            i = 16 + (m - 4) * 2 + kk
            w[:, i * P:(i + 1) * P] = blk.astype(np.float16)
    return np.ascontiguousarray(w)


def make_inputs(x, twiddle, bias):
    wB = make_weights(twiddle)
    coef = make_coeffs(twiddle, bias)
    x = np.asarray(x)
    in_maps = []
    for c in range(N_CORES):
        shard = x[c * SHARD:(c + 1) * SHARD]
        in_maps.append({
            "xT": np.ascontiguousarray(shard.T.astype(np.float16)),
            "wB": wB,
            "coef": coef,
        })
    return in_maps


def kernel(x: np.ndarray, twiddle: np.ndarray, bias: np.ndarray) -> np.ndarray:
    global _NC_CACHE
    if _NC_CACHE is None:
        _NC_CACHE = build_nc()
    nc = _NC_CACHE

    in_maps = make_inputs(x, twiddle, bias)
    res = run_bass_kernel_spmd(nc, in_maps, list(range(N_CORES)))
    out = np.empty((BATCH, N), dtype=np.float32)
    for c in range(N_CORES):
        out[c * SHARD:(c + 1) * SHARD] = res.results[c]["outT"].T.astype(np.float32)
    return out


# revision 10
# speedup vs baseline: 1.7255x; 1.3100x over previous
"""Butterfly network forward pass on 8 Trainium2 NeuronCores.

Strategy (v4, mixed-radix split + folded scales): stages 0-7 mix within
aligned 256-blocks, stage 8 within 512-blocks, stage 9 across them.

  LO half (features 0-511):  stages 0-8 fold into one 512x512 matrix ->
      GEMM contraction 512 (4 accumulated matmuls); PSUM holds the
      stage-8 output z_0..z_3 directly.
  HI half (features 512-1023): stages 0-7 fold into two 256x256 blocks,
      and the stage-8 "self" twiddle coefficient is folded into the
      weight COLUMNS host-side -> GEMM contraction 256 -> e_4..e_7;
      stage 8's cross term is a tensor_scalar (ratio coefficient) +
      tensor_tensor on the Vector engine.
  Stage 9 + bias (pairs z_m, z_{m+4} at fixed partition): u = (z_lo *
      t9a + bias) and v = (z_hi * t9b) as tensor_scalar, out = u + v.

PSUM is written in [128, 1024] two-bank tiles (batch chunks 2g, 2g+1 of
one feature chunk) so each ScalarE evacuation is a single plain Copy at
free-dim 1024 -- no per-partition scale needed anywhere in the epilogue.

Per-core approximate engine busy on HW: PE ~21us (96 matmuls), ACT
~16us (16 evacs), DVE ~19us (stage 8 HI + stage 9), DMA in/out streams
~13us each direction (overlapped).  Everything crosses the wire in fp16.
"""

import numpy as np

import concourse.bacc as bacc
import concourse.mybir as mybir
import concourse.tile as tile
from concourse.bass_utils import run_bass_kernel_spmd

N_CORES = 8
BATCH = 16384
N = 1024
M_STAGES = 10
SHARD = BATCH // N_CORES   # 2048 rows per core
P = 128                    # SBUF partitions
NB = 512                   # matmul moving-dim chunk (one PSUM bank fp32)
NBC = SHARD // NB          # 4 batch chunks per core
NCH = N // P               # 8 feature chunks of 128
PW = 1024                  # PSUM pair window (two banks) = evac free dim
FDS = 1024                 # free dim for the SBUF stage ops (>= PW)

F32 = mybir.dt.float32
F16 = mybir.dt.float16
MULT = mybir.AluOpType.mult
ADD = mybir.AluOpType.add

S8_PAIRS_HI = [(4, 6), (5, 7)]
NWBLK = 4 * 4 + 4 * 2      # 16 LO blocks + 8 HI blocks

_NC_CACHE = None

# knobs for A/B ablation (bench_ab passes these through)
BUILD_KW = {}


def build_nc(reps: int = 1, staggered: bool = True, hints: bool = True,
             fds: int = FDS, gpsimd_u: bool = False, warm: int = 16,
             mode: str = "full"):
    """Build the per-core kernel; reps>1 wraps the FULL body (including
    all input DMA) in a hardware loop for A/B timing by subtraction."""
    assert fds >= PW and fds % PW == 0
    nc = bacc.Bacc("TRN2", target_bir_lowering=False, debug=False,
                   num_devices=N_CORES)
    xT = nc.declare_dram_parameter("xT", [N, SHARD], F16, isOutput=False)
    wB = nc.declare_dram_parameter("wB", [P, NWBLK * P], F16, isOutput=False)
    # per-partition coefficient columns: [r8 | t9a | t9b | bias]
    coef = nc.declare_dram_parameter("coef", [P, 4 * NCH], F32, isOutput=False)
    outT = nc.declare_dram_parameter("outT", [N, SHARD], F16, isOutput=True)

    ngrp = SHARD // fds
    with tile.TileContext(nc) as tc:
        with (
            tc.tile_pool(name="wp", bufs=2) as wp,
            tc.tile_pool(name="xp", bufs=2) as xp,
            tc.tile_pool(name="cp", bufs=2) as cp,
            tc.tile_pool(name="pp", bufs=3, space="PSUM") as pp,
            tc.tile_pool(name="ppw", bufs=1, space="PSUM") as ppw,
            tc.tile_pool(name="ep", bufs=(2 if fds <= 1024 else 1)) as ep,
            tc.tile_pool(name="zp", bufs=(2 if fds <= 1024 else 1)) as zp,
            tc.tile_pool(name="up", bufs=(4 if fds <= 1024 else 2)) as up,
            tc.tile_pool(name="op", bufs=(8 if fds <= 1024 else 4)) as op,
        ):
            def body():
                ct = cp.tile([P, 4 * NCH], F32, tag="coef")
                nc.sync.dma_start(out=ct[:], in_=coef[:])
                r8 = ct[:, 0 * NCH:1 * NCH]
                t9a = ct[:, 1 * NCH:2 * NCH]
                t9b = ct[:, 2 * NCH:3 * NCH]
                bias8 = ct[:, 3 * NCH:4 * NCH]

                wt = wp.tile([P, NWBLK * P], F16, tag="w")
                nc.sync.dma_start(out=wt[:], in_=wB[:])

                def wblk(i):
                    return wt[:, i * P:(i + 1) * P]

                # x chunk tiles: column block k holds xT[k*P:(k+1)*P,
                # n*NB:(n+1)*NB]
                xsrc = xT.rearrange("(k p) (nb b) -> nb p k b", p=P, b=NB)
                xtiles = [xp.tile([P, NCH * NB], F16, tag=f"xc{n}",
                                  name=f"xc{n}") for n in range(NBC)]
                x0 = xtiles[0].rearrange("p (k b) -> p k b", b=NB)
                if mode != "componly":
                    for k in range(0, NCH, 2):
                        nc.sync.dma_start(out=x0[:, k:k + 2],
                                          in_=xsrc[0, :, k:k + 2])
                    for n in range(1, NBC):
                        dst = xtiles[n][:].rearrange("p (k b) -> p k b", b=NB)
                        h = NCH // 2
                        nc.sync.dma_start(out=dst[:, 0:h], in_=xsrc[n, :, 0:h])
                        nc.sync.dma_start(out=dst[:, h:NCH],
                                          in_=xsrc[n, :, h:NCH])
                else:
                    for n in range(NBC):
                        nc.vector.memset(xtiles[n][:, 0:1], 0.0)

                # Warm the PE (HAM clock gate) while the prologue streams.
                if warm:
                    wps = ppw.tile([NCH, NCH], F32, tag="warm")
                    for _ in range(warm):
                        nc.tensor.matmul(wps[:], lhsT=ct[:, 0:NCH],
                                         rhs=ct[:, 0:NCH], start=True,
                                         stop=True)

                if mode == "dmaonly":
                    dot = op.tile([P, fds], F16, tag="ot")
                    nc.vector.memset(dot[:], 0.0)
                    for m in range(NCH):
                        for g in range(ngrp):
                            nc.sync.dma_start(
                                out=outT[m * P:(m + 1) * P,
                                         g * fds:(g + 1) * fds],
                                in_=dot[:])
                    return

                for g in range(ngrp):
                    etiles = {}
                    for m in range(NCH):
                        etiles[m] = ep.tile([P, fds], F16,
                                            tag=f"e{m}", name=f"e{m}")
                    # GEMM in [P, PW] two-bank PSUM tiles: one feature
                    # chunk x two batch chunks, then one plain-copy evac.
                    for gp in range(fds // PW):
                        nlo = g * (fds // NB) + gp * (PW // NB)
                        for m in range(NCH):
                            ps = pp.tile([P, PW], F32, tag="ps")
                            for h, n in enumerate(range(nlo, nlo + PW // NB)):
                                xt = xtiles[n]
                                dst = ps[:, h * NB:(h + 1) * NB]
                                if m < 4:   # LO: contraction 512, stages 0-8
                                    for k in range(4):
                                        nc.tensor.matmul(
                                            dst, lhsT=wblk(m * 4 + k),
                                            rhs=xt[:, k * NB:(k + 1) * NB],
                                            start=(k == 0), stop=(k == 3))
                                else:       # HI: contraction 256, stages 0-7
                                    c = m // 2
                                    for kk in range(2):
                                        nc.tensor.matmul(
                                            dst,
                                            lhsT=wblk(16 + (m - 4) * 2 + kk),
                                            rhs=xt[:, (2 * c + kk) * NB:
                                                   (2 * c + kk + 1) * NB],
                                            start=(kk == 0), stop=(kk == 1))
                            nc.scalar.copy(
                                etiles[m][:, gp * PW:(gp + 1) * PW], ps[:])

                    # stage 8 cross terms on the HI half
                    ztiles = {}
                    for (a, b) in S8_PAIRS_HI:
                        ea, eb = etiles[a], etiles[b]
                        ta = up.tile([P, fds], F16, tag="t8")
                        nc.vector.tensor_scalar(ta[:], eb[:], r8[:, a:a + 1],
                                                None, op0=MULT)
                        za = zp.tile([P, fds], F16, tag=f"z{a}", name=f"z{a}")
                        nc.vector.tensor_tensor(za[:], ta[:], ea[:], op=ADD)
                        tb = up.tile([P, fds], F16, tag="t8")
                        nc.vector.tensor_scalar(tb[:], ea[:], r8[:, b:b + 1],
                                                None, op0=MULT)
                        zb = zp.tile([P, fds], F16, tag=f"z{b}", name=f"z{b}")
                        nc.vector.tensor_tensor(zb[:], tb[:], eb[:], op=ADD)
                        ztiles[a], ztiles[b] = za, zb

                    # stage 9 + bias
                    for m in range(NCH):
                        zlo = etiles[m % 4]
                        zhi = ztiles[m % 4 + 4]
                        ut = up.tile([P, fds], F16, tag="u")
                        ueng = nc.gpsimd if gpsimd_u else nc.vector
                        ueng.tensor_scalar(
                            ut[:], zlo[:], t9a[:, m:m + 1], bias8[:, m:m + 1],
                            op0=MULT, op1=ADD)
                        vt = up.tile([P, fds], F16, tag="v")
                        nc.vector.tensor_scalar(vt[:], zhi[:], t9b[:, m:m + 1],
                                                None, op0=MULT)
                        ot = op.tile([P, fds], F16, tag="ot")
                        nc.vector.tensor_tensor(ot[:], ut[:], vt[:], op=ADD)
                        if mode == "full":
                            nc.sync.dma_start(
                                out=outT[m * P:(m + 1) * P,
                                         g * fds:(g + 1) * fds],
                                in_=ot[:])

            if reps == 1:
                body()
            else:
                hint = (mybir.EngineType.PE,) if hints else ()
                with tc.For_i(0, reps, 1, hint_engines=hint,
                              staggered_reset=staggered):
                    body()
    nc.compile()
    return nc


def _clamp_den(v, eps=1e-20):
    s = np.where(v >= 0, 1.0, -1.0).astype(np.float32)
    return np.where(np.abs(v) < eps, s * eps, v).astype(np.float32)


def compose_stages(twiddle, hi) -> np.ndarray:
    """Fold stages 0..hi-1 into A^T = stages(I_N): [feat_in, feat_out]."""
    out = np.eye(N, dtype=np.float32)
    tw = np.asarray(twiddle, dtype=np.float32)
    for s in range(hi):
        stride = 1 << s
        nblk = N // (2 * stride)
        t = tw[0, s].reshape(nblk, stride, 2, 2)
        xr = out.reshape(N, nblk, 2, stride)
        out = np.einsum("krij,bkjr->bkir", t, xr,
                        dtype=np.float32).reshape(N, N)
    return out


def _s8_coeffs(twiddle):
    """Stage-8 per-(chunk, partition) coefficients for the HI half."""
    tw = np.asarray(twiddle, dtype=np.float32)
    t8 = tw[0, 8].reshape(2, 256, 2, 2)   # [k8, r, i, j]
    p = np.arange(P)
    alpha = np.ones((P, NCH), np.float32)
    r8 = np.zeros((P, NCH), np.float32)
    for (a, b) in S8_PAIRS_HI:
        k8 = a // 4                       # == 1
        r = (a % 2) * 128 + p
        t00 = _clamp_den(t8[k8, r, 0, 0])
        t01 = t8[k8, r, 0, 1]
        t10 = t8[k8, r, 1, 0]
        t11 = _clamp_den(t8[k8, r, 1, 1])
        alpha[:, a] = t00
        alpha[:, b] = t11
        r8[:, a] = t01 / t11              # z_a = e_a + r8_a * e_b
        r8[:, b] = t10 / t00              # z_b = e_b + r8_b * e_a
    return alpha, r8


def make_coeffs(twiddle, bias):
    """Per-partition coefficient table [P, 4*NCH] fp32:
    columns [r8 | t9a | t9b | bias]."""
    tw = np.asarray(twiddle, dtype=np.float32)
    t9 = tw[0, 9].reshape(512, 2, 2)      # [r, i, j]
    p = np.arange(P)
    _, r8 = _s8_coeffs(twiddle)
    t9a = np.zeros((P, NCH), np.float32)
    t9b = np.zeros((P, NCH), np.float32)
    for m in range(NCH):
        i = m // 4
        r = (m % 4) * 128 + p
        t9a[:, m] = t9[r, i, 0]           # coefficient of z_{m%4}
        t9b[:, m] = t9[r, i, 1]           # coefficient of z_{m%4+4}
    bias8 = np.asarray(bias, np.float32).reshape(NCH, P).T
    return np.ascontiguousarray(
        np.concatenate([r8, t9a, t9b, bias8], axis=1))


def make_weights(twiddle) -> np.ndarray:
    """lhsT blocks packed as [P, NWBLK*P] fp16: 16 LO blocks (contraction
    512, stages 0-8) then 8 HI blocks (contraction 256, stages 0-7 with
    the stage-8 self coefficient folded into the columns)."""
    aT8 = compose_stages(twiddle, 8)      # [in, out], 256-block diagonal
    aT9 = compose_stages(twiddle, 9)      # [in, out], 512-block diagonal
    alpha, _ = _s8_coeffs(twiddle)
    w = np.empty((P, NWBLK * P), np.float16)
    for m in range(4):                    # LO out-chunks
        for k in range(4):
            blk = aT9[k * P:(k + 1) * P, m * P:(m + 1) * P]
            w[:, (m * 4 + k) * P:(m * 4 + k + 1) * P] = blk.astype(np.float16)
    for m in range(4, NCH):               # HI out-chunks, alpha-scaled cols
        c = m // 2
        for kk in range(2):
            fin = c * 256 + kk * P
            blk = aT8[fin:fin + P, m * P:(m + 1) * P] * alpha[:, m][None, :]
            i = 16 + (m - 4) * 2 + kk
            w[:, i * P:(i + 1) * P] = blk.astype(np.float16)
    return np.ascontiguousarray(w)


def make_inputs(x, twiddle, bias):
    wB = make_weights(twiddle)
    coef = make_coeffs(twiddle, bias)
    x = np.asarray(x)
    in_maps = []
    for c in range(N_CORES):
        shard = x[c * SHARD:(c + 1) * SHARD]
        in_maps.append({
            "xT": np.ascontiguousarray(shard.T.astype(np.float16)),
            "wB": wB,
            "coef": coef,
        })
    return in_maps


def kernel(x: np.ndarray, twiddle: np.ndarray, bias: np.ndarray) -> np.ndarray:
    global _NC_CACHE
    if _NC_CACHE is None:
        _NC_CACHE = build_nc()
    nc = _NC_CACHE

    in_maps = make_inputs(x, twiddle, bias)
    res = run_bass_kernel_spmd(nc, in_maps, list(range(N_CORES)))
    out = np.empty((BATCH, N), dtype=np.float32)
    for c in range(N_CORES):
        out[c * SHARD:(c + 1) * SHARD] = res.results[c]["outT"].T.astype(np.float32)
    return out


# revision 11
# speedup vs baseline: 1.7831x; 1.0334x over previous
"""Butterfly network forward pass on 8 Trainium2 NeuronCores.

Strategy (v4, mixed-radix split + folded scales): stages 0-7 mix within
aligned 256-blocks, stage 8 within 512-blocks, stage 9 across them.

  LO half (features 0-511):  stages 0-8 fold into one 512x512 matrix ->
      GEMM contraction 512 (4 accumulated matmuls); PSUM holds the
      stage-8 output z_0..z_3 directly.
  HI half (features 512-1023): stages 0-7 fold into two 256x256 blocks,
      and the stage-8 "self" twiddle coefficient is folded into the
      weight COLUMNS host-side -> GEMM contraction 256 -> e_4..e_7;
      stage 8's cross term is a tensor_scalar (ratio coefficient) +
      tensor_tensor on the Vector engine.
  Stage 9 + bias (pairs z_m, z_{m+4} at fixed partition): u = (z_lo *
      t9a + bias) and v = (z_hi * t9b) as tensor_scalar, out = u + v.

PSUM is written in [128, 1024] two-bank tiles (batch chunks 2g, 2g+1 of
one feature chunk) so each ScalarE evacuation is a single plain Copy at
free-dim 1024 -- no per-partition scale needed anywhere in the epilogue.

Per-core approximate engine busy on HW: PE ~21us (96 matmuls), ACT
~16us (16 evacs), DVE ~19us (stage 8 HI + stage 9), DMA in/out streams
~13us each direction (overlapped).  Everything crosses the wire in fp16.
"""

import numpy as np

import concourse.bacc as bacc
import concourse.mybir as mybir
import concourse.tile as tile
from concourse.bass_utils import run_bass_kernel_spmd

N_CORES = 8
BATCH = 16384
N = 1024
M_STAGES = 10
SHARD = BATCH // N_CORES   # 2048 rows per core
P = 128                    # SBUF partitions
NB = 512                   # matmul moving-dim chunk (one PSUM bank fp32)
NBC = SHARD // NB          # 4 batch chunks per core
NCH = N // P               # 8 feature chunks of 128
PW = 1024                  # PSUM pair window (two banks) = evac free dim
FDS = 1024                 # free dim for the SBUF stage ops (>= PW)

F32 = mybir.dt.float32
F16 = mybir.dt.float16
MULT = mybir.AluOpType.mult
ADD = mybir.AluOpType.add

S8_PAIRS_HI = [(4, 6), (5, 7)]
S8_PAIRS_ALL = [(0, 2), (1, 3), (4, 6), (5, 7)]
NWBLK = 4 * 4 + 4 * 2 + 4 * 2   # 16 LO-512 blocks + 8 HI + 8 LO-256 blocks

_NC_CACHE = None

# knobs for A/B ablation (bench_ab passes these through)
BUILD_KW = {}


def build_nc(reps: int = 1, staggered: bool = True, hints: bool = True,
             fds: int = FDS, gpsimd_u: bool = False, warm: int = 16,
             mode: str = "full", k2all: bool = False):
    """Build the per-core kernel; reps>1 wraps the FULL body (including
    all input DMA) in a hardware loop for A/B timing by subtraction."""
    assert fds >= PW and fds % PW == 0
    nc = bacc.Bacc("TRN2", target_bir_lowering=False, debug=False,
                   num_devices=N_CORES)
    xT = nc.declare_dram_parameter("xT", [N, SHARD], F16, isOutput=False)
    wB = nc.declare_dram_parameter("wB", [P, NWBLK * P], F16, isOutput=False)
    # per-partition coefficient columns: [r8 | t9a | t9b | bias]
    coef = nc.declare_dram_parameter("coef", [P, 4 * NCH], F32, isOutput=False)
    outT = nc.declare_dram_parameter("outT", [N, SHARD], F16, isOutput=True)

    ngrp = SHARD // fds
    with tile.TileContext(nc) as tc:
        with (
            tc.tile_pool(name="wp", bufs=2) as wp,
            tc.tile_pool(name="xp", bufs=2) as xp,
            tc.tile_pool(name="cp", bufs=2) as cp,
            tc.tile_pool(name="pp", bufs=3, space="PSUM") as pp,
            tc.tile_pool(name="ppw", bufs=1, space="PSUM") as ppw,
            tc.tile_pool(name="ep", bufs=(2 if fds <= 1024 else 1)) as ep,
            tc.tile_pool(name="zp", bufs=(2 if fds <= 1024 else 1)) as zp,
            tc.tile_pool(name="up", bufs=(4 if fds <= 1024 else 2)) as up,
            tc.tile_pool(name="op", bufs=(8 if fds <= 1024 else 4)) as op,
        ):
            def body():
                ct = cp.tile([P, 4 * NCH], F32, tag="coef")
                nc.sync.dma_start(out=ct[:], in_=coef[:])
                r8 = ct[:, 0 * NCH:1 * NCH]
                t9a = ct[:, 1 * NCH:2 * NCH]
                t9b = ct[:, 2 * NCH:3 * NCH]
                bias8 = ct[:, 3 * NCH:4 * NCH]

                wt = wp.tile([P, NWBLK * P], F16, tag="w")
                nc.sync.dma_start(out=wt[:], in_=wB[:])

                def wblk(i):
                    return wt[:, i * P:(i + 1) * P]

                # x chunk tiles: column block k holds xT[k*P:(k+1)*P,
                # n*NB:(n+1)*NB]
                xsrc = xT.rearrange("(k p) (nb b) -> nb p k b", p=P, b=NB)
                xtiles = [xp.tile([P, NCH * NB], F16, tag=f"xc{n}",
                                  name=f"xc{n}") for n in range(NBC)]
                x0 = xtiles[0].rearrange("p (k b) -> p k b", b=NB)
                if mode != "componly":
                    for k in range(0, NCH, 2):
                        nc.sync.dma_start(out=x0[:, k:k + 2],
                                          in_=xsrc[0, :, k:k + 2])
                    for n in range(1, NBC):
                        dst = xtiles[n][:].rearrange("p (k b) -> p k b", b=NB)
                        h = NCH // 2
                        nc.sync.dma_start(out=dst[:, 0:h], in_=xsrc[n, :, 0:h])
                        nc.sync.dma_start(out=dst[:, h:NCH],
                                          in_=xsrc[n, :, h:NCH])
                else:
                    for n in range(NBC):
                        nc.vector.memset(xtiles[n][:, 0:1], 0.0)

                # Warm the PE (HAM clock gate) while the prologue streams.
                if warm:
                    wps = ppw.tile([NCH, NCH], F32, tag="warm")
                    for _ in range(warm):
                        nc.tensor.matmul(wps[:], lhsT=ct[:, 0:NCH],
                                         rhs=ct[:, 0:NCH], start=True,
                                         stop=True)

                if mode == "dmaonly":
                    dot = op.tile([P, fds], F16, tag="ot")
                    nc.vector.memset(dot[:], 0.0)
                    for m in range(NCH):
                        for g in range(ngrp):
                            nc.sync.dma_start(
                                out=outT[m * P:(m + 1) * P,
                                         g * fds:(g + 1) * fds],
                                in_=dot[:])
                    return

                for g in range(ngrp):
                    etiles = {}
                    for m in range(NCH):
                        etiles[m] = ep.tile([P, fds], F16,
                                            tag=f"e{m}", name=f"e{m}")
                    # GEMM in [P, PW] two-bank PSUM tiles: one feature
                    # chunk x two batch chunks, then one plain-copy evac.
                    for gp in range(fds // PW):
                        nlo = g * (fds // NB) + gp * (PW // NB)
                        for m in range(NCH):
                            ps = pp.tile([P, PW], F32, tag="ps")
                            for h, n in enumerate(range(nlo, nlo + PW // NB)):
                                xt = xtiles[n]
                                dst = ps[:, h * NB:(h + 1) * NB]
                                if m < 4 and not k2all:
                                    # LO: contraction 512, stages 0-8
                                    for k in range(4):
                                        nc.tensor.matmul(
                                            dst, lhsT=wblk(m * 4 + k),
                                            rhs=xt[:, k * NB:(k + 1) * NB],
                                            start=(k == 0), stop=(k == 3))
                                elif m < 4:
                                    # k2all LO: contraction 256, stages 0-7
                                    c = m // 2
                                    for kk in range(2):
                                        nc.tensor.matmul(
                                            dst,
                                            lhsT=wblk(24 + m * 2 + kk),
                                            rhs=xt[:, (2 * c + kk) * NB:
                                                   (2 * c + kk + 1) * NB],
                                            start=(kk == 0), stop=(kk == 1))
                                else:       # HI: contraction 256, stages 0-7
                                    c = m // 2
                                    for kk in range(2):
                                        nc.tensor.matmul(
                                            dst,
                                            lhsT=wblk(16 + (m - 4) * 2 + kk),
                                            rhs=xt[:, (2 * c + kk) * NB:
                                                   (2 * c + kk + 1) * NB],
                                            start=(kk == 0), stop=(kk == 1))
                            nc.scalar.copy(
                                etiles[m][:, gp * PW:(gp + 1) * PW], ps[:])

                    # stage 8 cross terms
                    pairs = (S8_PAIRS_ALL if k2all else S8_PAIRS_HI)
                    ztiles = {}
                    for (a, b) in pairs:
                        ea, eb = etiles[a], etiles[b]
                        ta = up.tile([P, fds], F16, tag="t8")
                        nc.vector.tensor_scalar(ta[:], eb[:], r8[:, a:a + 1],
                                                None, op0=MULT)
                        za = zp.tile([P, fds], F16, tag=f"z{a}", name=f"z{a}")
                        nc.vector.tensor_tensor(za[:], ta[:], ea[:], op=ADD)
                        tb = up.tile([P, fds], F16, tag="t8")
                        nc.vector.tensor_scalar(tb[:], ea[:], r8[:, b:b + 1],
                                                None, op0=MULT)
                        zb = zp.tile([P, fds], F16, tag=f"z{b}", name=f"z{b}")
                        nc.vector.tensor_tensor(zb[:], tb[:], eb[:], op=ADD)
                        ztiles[a], ztiles[b] = za, zb

                    # stage 9 + bias
                    for m in range(NCH):
                        zlo = ztiles[m % 4] if k2all else etiles[m % 4]
                        zhi = ztiles[m % 4 + 4]
                        ut = up.tile([P, fds], F16, tag="u")
                        ueng = nc.gpsimd if gpsimd_u else nc.vector
                        ueng.tensor_scalar(
                            ut[:], zlo[:], t9a[:, m:m + 1], bias8[:, m:m + 1],
                            op0=MULT, op1=ADD)
                        vt = up.tile([P, fds], F16, tag="v")
                        nc.vector.tensor_scalar(vt[:], zhi[:], t9b[:, m:m + 1],
                                                None, op0=MULT)
                        ot = op.tile([P, fds], F16, tag="ot")
                        nc.vector.tensor_tensor(ot[:], ut[:], vt[:], op=ADD)
                        if mode == "full":
                            nc.sync.dma_start(
                                out=outT[m * P:(m + 1) * P,
                                         g * fds:(g + 1) * fds],
                                in_=ot[:])

            if reps == 1:
                body()
            else:
                hint = (mybir.EngineType.PE,) if hints else ()
                with tc.For_i(0, reps, 1, hint_engines=hint,
                              staggered_reset=staggered):
                    body()
    nc.compile()
    return nc


def _clamp_den(v, eps=1e-20):
    s = np.where(v >= 0, 1.0, -1.0).astype(np.float32)
    return np.where(np.abs(v) < eps, s * eps, v).astype(np.float32)


def compose_stages(twiddle, hi) -> np.ndarray:
    """Fold stages 0..hi-1 into A^T = stages(I_N): [feat_in, feat_out]."""
    out = np.eye(N, dtype=np.float32)
    tw = np.asarray(twiddle, dtype=np.float32)
    for s in range(hi):
        stride = 1 << s
        nblk = N // (2 * stride)
        t = tw[0, s].reshape(nblk, stride, 2, 2)
        xr = out.reshape(N, nblk, 2, stride)
        out = np.einsum("krij,bkjr->bkir", t, xr,
                        dtype=np.float32).reshape(N, N)
    return out


def _s8_coeffs(twiddle):
    """Stage-8 per-(chunk, partition) coefficients for the HI half."""
    tw = np.asarray(twiddle, dtype=np.float32)
    t8 = tw[0, 8].reshape(2, 256, 2, 2)   # [k8, r, i, j]
    p = np.arange(P)
    alpha = np.ones((P, NCH), np.float32)
    r8 = np.zeros((P, NCH), np.float32)
    for (a, b) in S8_PAIRS_ALL:
        k8 = a // 4
        r = (a % 2) * 128 + p
        t00 = _clamp_den(t8[k8, r, 0, 0])
        t01 = t8[k8, r, 0, 1]
        t10 = t8[k8, r, 1, 0]
        t11 = _clamp_den(t8[k8, r, 1, 1])
        alpha[:, a] = t00
        alpha[:, b] = t11
        r8[:, a] = t01 / t11              # z_a = e_a + r8_a * e_b
        r8[:, b] = t10 / t00              # z_b = e_b + r8_b * e_a
    return alpha, r8


def make_coeffs(twiddle, bias):
    """Per-partition coefficient table [P, 4*NCH] fp32:
    columns [r8 | t9a | t9b | bias]."""
    tw = np.asarray(twiddle, dtype=np.float32)
    t9 = tw[0, 9].reshape(512, 2, 2)      # [r, i, j]
    p = np.arange(P)
    _, r8 = _s8_coeffs(twiddle)
    t9a = np.zeros((P, NCH), np.float32)
    t9b = np.zeros((P, NCH), np.float32)
    for m in range(NCH):
        i = m // 4
        r = (m % 4) * 128 + p
        t9a[:, m] = t9[r, i, 0]           # coefficient of z_{m%4}
        t9b[:, m] = t9[r, i, 1]           # coefficient of z_{m%4+4}
    bias8 = np.asarray(bias, np.float32).reshape(NCH, P).T
    return np.ascontiguousarray(
        np.concatenate([r8, t9a, t9b, bias8], axis=1))


def make_weights(twiddle) -> np.ndarray:
    """lhsT blocks packed as [P, NWBLK*P] fp16: 16 LO blocks (contraction
    512, stages 0-8) then 8 HI blocks (contraction 256, stages 0-7 with
    the stage-8 self coefficient folded into the columns)."""
    aT8 = compose_stages(twiddle, 8)      # [in, out], 256-block diagonal
    aT9 = compose_stages(twiddle, 9)      # [in, out], 512-block diagonal
    alpha, _ = _s8_coeffs(twiddle)
    w = np.empty((P, NWBLK * P), np.float16)
    for m in range(4):                    # LO out-chunks
        for k in range(4):
            blk = aT9[k * P:(k + 1) * P, m * P:(m + 1) * P]
            w[:, (m * 4 + k) * P:(m * 4 + k + 1) * P] = blk.astype(np.float16)
    for m in range(4, NCH):               # HI out-chunks, alpha-scaled cols
        c = m // 2
        for kk in range(2):
            fin = c * 256 + kk * P
            blk = aT8[fin:fin + P, m * P:(m + 1) * P] * alpha[:, m][None, :]
            i = 16 + (m - 4) * 2 + kk
            w[:, i * P:(i + 1) * P] = blk.astype(np.float16)
    for m in range(4):                    # k2all LO-256 blocks, alpha-scaled
        c = m // 2
        for kk in range(2):
            fin = c * 256 + kk * P
            blk = aT8[fin:fin + P, m * P:(m + 1) * P] * alpha[:, m][None, :]
            i = 24 + m * 2 + kk
            w[:, i * P:(i + 1) * P] = blk.astype(np.float16)
    return np.ascontiguousarray(w)


def make_inputs(x, twiddle, bias):
    wB = make_weights(twiddle)
    coef = make_coeffs(twiddle, bias)
    x = np.asarray(x)
    in_maps = []
    for c in range(N_CORES):
        shard = x[c * SHARD:(c + 1) * SHARD]
        in_maps.append({
            "xT": np.ascontiguousarray(shard.T.astype(np.float16)),
            "wB": wB,
            "coef": coef,
        })
    return in_maps


def kernel(x: np.ndarray, twiddle: np.ndarray, bias: np.ndarray) -> np.ndarray:
    global _NC_CACHE
    if _NC_CACHE is None:
        _NC_CACHE = build_nc()
    nc = _NC_CACHE

    in_maps = make_inputs(x, twiddle, bias)
    res = run_bass_kernel_spmd(nc, in_maps, list(range(N_CORES)))
    out = np.empty((BATCH, N), dtype=np.float32)
    for c in range(N_CORES):
        out[c * SHARD:(c + 1) * SHARD] = res.results[c]["outT"].T.astype(np.float32)
    return out
